# revision 21
# baseline (speedup 1.0000x reference)
"""Instant-NGP style hash encoding on 8 trn2 NeuronCores.

Wall-time-optimized: the axon tunnel moves ~55MB/s, so the kernel
minimizes host<->device bytes.
 - Tables are converted to bf16, packed flat, uploaded SHARDED (1/8 per
   core) once, AllGathered on device, and kept device-resident across
   calls (content-checked against the previous call's arrays).
 - Dense grids are expanded to per-cell 8-corner blocks (16 bf16 = 32B
   contiguous) so each dense point-level costs one gather descriptor.
 - x is uploaded f32 (24MB); output is quantized on device to uint8
   with a per-tile per-partition scale (60MB down + 0.5MB scales)
   and dequantized on host.
 - Donated output buffers are created on device (jnp.zeros under jit),
   not uploaded.
Device kernel: point-parallel, each core 262144 points, all 15 levels;
per level DVE computes corner indices + trilinear weights; corner values
fetched with [128,1]-offset indirect DMAs (one offset per partition per
instruction) through static staging tiles; MAC via elementwise mult +
tensor_reduce.
"""
import sys
sys.path.insert(0, '/opt/trn_rl_repo')
import numpy as np

N = 2097152
NC = 8
NSHARD = N // NC          # 262144 points per core
F = 128                   # free-dim points per partition per tile
PTILE = 128 * F           # 16384 points per tile
NT = NSHARD // PTILE      # 16 tiles per core
GRID_SIZES = [16, 23, 32, 45, 64, 91, 128, 181, 256, 362, 512, 724, 1024, 1448, 2048]
HASH_MAP_SIZE = 2 ** 19
P2 = 2654435761
P3 = 805459861
MASK = HASH_MAP_SIZE - 1

DENSE_GS = [g for g in GRID_SIZES if g ** 3 <= HASH_MAP_SIZE]    # [16,23,32,45,64]
HASH_GS = [g for g in GRID_SIZES if g ** 3 > HASH_MAP_SIZE]      # 10 levels

# --- packed bf16 table layout (element = bf16 scalar) ---
# hash level j: 524288 rows x 2 feats at row2-base j*524288
HASH_ROW2_BASE = {gs: j * HASH_MAP_SIZE for j, gs in enumerate(HASH_GS)}
_e = len(HASH_GS) * HASH_MAP_SIZE * 2          # 20971520 elements
DENSE_ROW16_BASE = {}
for _G in DENSE_GS:
    DENSE_ROW16_BASE[_G] = _e // 16
    _e += (_G - 1) ** 3 * 16
E_TOT = _e                                      # 27036240
E_PAD = ((E_TOT + 127) // 128) * 128            # 27036288
ES = E_PAD // NC                                # per-core shard elements

_S = {}


def _build_gather_nc():
    """NEFF-A: AllGather the bf16 table shard into the full table."""
    from concourse import bacc
    import concourse.mybir as mybir
    import concourse.tile as tile
    bf16 = mybir.dt.bfloat16

    nc = bacc.Bacc("TRN2", target_bir_lowering=False, debug=False, num_devices=NC)
    tshard = nc.dram_tensor("tshard", [ES], bf16, kind="ExternalInput")
    tabs = nc.dram_tensor("tabs", [E_PAD], bf16, kind="ExternalOutput")
    with tile.TileContext(nc) as tc:
        with tc.tile_pool(name="d", bufs=1, space="DRAM") as dram:
            bin_ = dram.tile([ES], bf16)
            bout = dram.tile([E_PAD], bf16)
            nc.gpsimd.dma_start(bin_[:], tshard.ap())
            nc.gpsimd.collective_compute(
                "AllGather", mybir.AluOpType.bypass,
                replica_groups=[list(range(NC))],
                ins=[bin_.opt()], outs=[bout.opt()],
            )
            nc.gpsimd.dma_start(tabs.ap(), bout[:])
    nc.compile()
    return nc


def _build_main_nc():
    from concourse import bacc
    import concourse.bass as bass
    import concourse.mybir as mybir
    import concourse.tile as tile

    f32 = mybir.dt.float32
    i32 = mybir.dt.int32
    bf16 = mybir.dt.bfloat16
    u8 = mybir.dt.uint8
    Alu = mybir.AluOpType

    nc = bacc.Bacc("TRN2", target_bir_lowering=False, debug=False, num_devices=NC)

    x_in = nc.dram_tensor("x", [NSHARD, 3], f32, kind="ExternalInput")
    tabs = nc.dram_tensor("tabs", [E_PAD], bf16, kind="ExternalInput")
    # outputs are AllGathered on-device so every core holds the full result:
    # the host then does a single-buffer fetch (the multi-shard D2H path is
    # much slower on the axon tunnel)
    outq = nc.dram_tensor("outq", [N, 30], u8, kind="ExternalOutput")
    amax_o = nc.dram_tensor("amax_o", [NC * NT, 128, 1], f32, kind="ExternalOutput")

    tab2 = tabs.ap().rearrange("(r c) -> r c", c=2)     # hash rows
    # dense 8-corner blocks: gather as 8 x f32 (32B) rows via bitcast — the
    # HW indirect-DMA path is only reliable with flat dest APs and this row
    # shape (bf16 16-wide rows come back corrupted).
    tabD = tabs.ap().bitcast(f32).rearrange("(r c) -> r c", c=8)

    x_v = x_in.ap().rearrange("(t p f) c -> t p (f c)", t=NT, p=128, f=F)

    with tile.TileContext(nc) as tc:
        with tc.tile_pool(name="main", bufs=2) as pool, \
             tc.tile_pool(name="stage", bufs=2) as spool, \
             tc.tile_pool(name="dram", bufs=1, space="DRAM") as dpool:
            oq_local = dpool.tile([NSHARD * 30], u8)
            am_local = dpool.tile([NT * 128], f32)
            oq_full = dpool.tile([N * 30], u8)
            am_full = dpool.tile([NC * NT * 128], f32)
            out_v = oq_local[:].rearrange("(t p f) -> t p f", t=NT, p=128, f=F * 30)
            am_v = am_local[:].rearrange("(t p u) -> t p u", t=NT, p=128, u=1)

            def process_tile(t_iv):
                xt = pool.tile([128, F * 3], f32, tag="xt")
                nc.sync.dma_start(xt[:], x_v[t_iv, :, :])
                oacc = pool.tile([128, F, 30], f32, tag="oacc")

                # deinterleave and normalize: xn = (x + 2) * 0.25
                xn = []
                for d in range(3):
                    xd = pool.tile([128, F], f32, tag=f"xn{d}")
                    nc.vector.tensor_scalar(xd[:], xt[:].rearrange("p (f c) -> p f c", c=3)[:, :, d], 2.0, None, Alu.add)
                    nc.vector.tensor_scalar(xd[:], xd[:], 0.25, None, Alu.mult)
                    xn.append(xd)

                for li, gs in enumerate(GRID_SIZES):
                    dense = gs ** 3 <= HASH_MAP_SIZE
                    # --- per-dim: u, floor -> (bi, bf), frac t ---
                    b_i, b_f, t_f = [], [], []
                    for d in range(3):
                        u = pool.tile([128, F], f32, tag=f"u{d}")
                        nc.vector.tensor_scalar(u[:], xn[d][:], float(gs), None, Alu.mult)
                        nc.vector.tensor_scalar(u[:], u[:], 0.5, None, Alu.subtract)
                        # floor(u): b0 = cast(u); fix = (float(b0) > u); b = b0 - fix
                        bi = pool.tile([128, F], i32, tag=f"bi{d}")
                        nc.vector.tensor_copy(bi[:], u[:])
                        bf = pool.tile([128, F], f32, tag=f"bf{d}")
                        nc.vector.tensor_copy(bf[:], bi[:])
                        fixi = pool.tile([128, F], i32, tag=f"fxi{d}")
                        nc.vector.tensor_tensor(fixi[:], bf[:], u[:], Alu.is_gt)
                        fixf = pool.tile([128, F], f32, tag=f"fxf{d}")
                        nc.vector.tensor_copy(fixf[:], fixi[:])
                        nc.vector.tensor_tensor(bi[:], bi[:], fixi[:], Alu.subtract)
                        nc.vector.tensor_tensor(bf[:], bf[:], fixf[:], Alu.subtract)
                        tf = pool.tile([128, F], f32, tag=f"tf{d}")
                        nc.vector.tensor_tensor(tf[:], u[:], bf[:], Alu.subtract)
                        b_i.append(bi)
                        b_f.append(bf)
                        t_f.append(tf)

                    if dense:
                        G = gs
                        # clamp-folded pair weights per dim:
                        # whi = t, but 0 if b<0, 1 if b>G-2 ; wlo = 1-whi
                        whi, wlo = [], []
                        for d in range(3):
                            mneg = pool.tile([128, F], f32, tag=f"mneg{d}")
                            nc.vector.tensor_scalar(mneg[:], b_f[d][:], 0.0, None, Alu.is_lt)
                            mhi = pool.tile([128, F], f32, tag=f"mhi{d}")
                            nc.vector.tensor_scalar(mhi[:], b_f[d][:], float(G - 2), None, Alu.is_gt)
                            wh = pool.tile([128, F], f32, tag=f"wh{d}")
                            # wh = t*(1-mneg)*(1-mhi) + mhi
                            nc.vector.tensor_scalar(mneg[:], mneg[:], -1.0, 1.0, Alu.mult, Alu.add)  # 1-mneg
                            nc.vector.tensor_tensor(wh[:], t_f[d][:], mneg[:], Alu.mult)
                            tmneg = pool.tile([128, F], f32, tag=f"tmneg{d}")
                            nc.vector.tensor_scalar(tmneg[:], mhi[:], -1.0, 1.0, Alu.mult, Alu.add)  # 1-mhi
                            nc.vector.tensor_tensor(wh[:], wh[:], tmneg[:], Alu.mult)
                            nc.vector.tensor_tensor(wh[:], wh[:], mhi[:], Alu.add)
                            wl = pool.tile([128, F], f32, tag=f"wl{d}")
                            nc.vector.tensor_scalar(wl[:], wh[:], -1.0, 1.0, Alu.mult, Alu.add)
                            whi.append(wh)
                            wlo.append(wl)
                        # block coords bc = clip(b, 0, G-2); idx = (bz*(G-1)+by)*(G-1)+bx + base
                        bc = []
                        for d in range(3):
                            c0 = pool.tile([128, F], i32, tag=f"bc{d}")
                            nc.vector.tensor_scalar(c0[:], b_i[d][:], 0, None, Alu.max)
                            nc.vector.tensor_scalar(c0[:], c0[:], G - 2, None, Alu.min)
                            bc.append(c0)
                        idxD = pool.tile([128, F], i32, tag="idxD")
                        nc.vector.tensor_scalar(idxD[:], bc[2][:], G - 1, None, Alu.mult)
                        nc.vector.tensor_tensor(idxD[:], idxD[:], bc[1][:], Alu.add)
                        nc.vector.tensor_scalar(idxD[:], idxD[:], G - 1, None, Alu.mult)
                        nc.vector.tensor_tensor(idxD[:], idxD[:], bc[0][:], Alu.add)
                        nc.vector.tensor_scalar(idxD[:], idxD[:], DENSE_ROW16_BASE[G], None, Alu.add)

                        # gather 32B blocks: F instructions of [128,1] offsets
                        vD0 = pool.tile([128, F, 8], f32, tag="vD0")
                        vD1 = pool.tile([128, F, 8], f32, tag="vD1")
                        CHD = 32

                        def gbodyD(j_iv):
                            isg = spool.tile([128, CHD], i32, tag="isgD")
                            vsg = spool.tile([128, CHD, 8], f32, tag="vsgD")
                            nc.vector.tensor_copy(isg[:], idxD[:, bass.ds(j_iv, CHD)])
                            for m in range(CHD):
                                nc.gpsimd.indirect_dma_start(
                                    out=vsg[:, m, :], out_offset=None, in_=tabD,
                                    in_offset=bass.IndirectOffsetOnAxis(ap=isg[:, m:m + 1], axis=0),
                                )
                            vb = vsg[:].bitcast(bf16).rearrange("p k (a two) -> p k a two", two=2)
                            nc.scalar.copy(vD0[:, bass.ds(j_iv, CHD), :], vb[:, :, :, 0])
                            nc.scalar.copy(vD1[:, bass.ds(j_iv, CHD), :], vb[:, :, :, 1])

                        tc.For_i_unrolled(0, F, CHD, gbodyD, max_unroll=2)

                        # 8 corner weights: c = 4dz+2dy+dx, w = (wz*wy)*wx
                        w_l = pool.tile([128, F, 8], f32, tag="w_l")
                        w01 = pool.tile([128, F], f32, tag="w01")
                        for dz in range(2):
                            wz = whi[2] if dz else wlo[2]
                            for dy in range(2):
                                wy = whi[1] if dy else wlo[1]
                                nc.vector.tensor_tensor(w01[:], wz[:], wy[:], Alu.mult)
                                for dx in range(2):
                                    wx = whi[0] if dx else wlo[0]
                                    c = 4 * dz + 2 * dy + dx
                                    nc.vector.tensor_tensor(w_l[:, :, c], w01[:], wx[:], Alu.mult)

                        prod = pool.tile([128, F, 8], f32, tag="prod")
                        for k, vv in ((0, vD0), (1, vD1)):
                            nc.vector.tensor_tensor(prod[:], w_l[:], vv[:], Alu.mult)
                            nc.vector.tensor_reduce(oacc[:, :, 2 * li + k], prod[:],
                                                    mybir.AxisListType.X, Alu.add)
                        continue

                    # ---------- hash level ----------
                    # idx = (x ^ y*P2 ^ z*P3) & MASK per corner; c = 4dx+2dy+dz
                    idx_l = pool.tile([128, F, 8], i32, tag="idx_l")
                    xs = []
                    for dx in range(2):
                        xm = pool.tile([128, F], i32, tag=f"hx{dx}")
                        if dx == 0:
                            nc.vector.tensor_scalar(xm[:], b_i[0][:], MASK, None, Alu.bitwise_and)
                        else:
                            nc.vector.tensor_scalar(xm[:], b_i[0][:], 1, None, Alu.add)
                            nc.vector.tensor_scalar(xm[:], xm[:], MASK, None, Alu.bitwise_and)
                        xs.append(xm)
                    hy, hz = [], []
                    piece = pool.tile([128, F], i32, tag="hpiece")
                    prodh = pool.tile([128, F], i32, tag="hprod")
                    for (dst, prime, src) in ((hy, P2, b_i[1]), (hz, P3, b_i[2])):
                        C = [(prime << (5 * s)) % HASH_MAP_SIZE for s in range(3)]
                        yq = pool.tile([128, F], i32, tag=f"yq{prime}")
                        nc.vector.tensor_scalar(yq[:], src[:], 1, None, Alu.add)  # in [0, 2049]
                        acc = pool.tile([128, F], i32, tag=f"hacc{prime}")
                        for s in range(3):
                            if s == 0:
                                nc.vector.tensor_scalar(piece[:], yq[:], 31, None, Alu.bitwise_and)
                            else:
                                nc.vector.tensor_scalar(piece[:], yq[:], 5 * s, None, Alu.logical_shift_right)
                                if s == 1:
                                    nc.vector.tensor_scalar(piece[:], piece[:], 31, None, Alu.bitwise_and)
                            tgt = acc if s == 0 else prodh
                            nc.vector.tensor_scalar(tgt[:], piece[:], C[s], None, Alu.mult)
                            nc.vector.tensor_scalar(tgt[:], tgt[:], MASK, None, Alu.bitwise_and)
                            if s > 0:
                                nc.vector.tensor_tensor(acc[:], acc[:], prodh[:], Alu.add)
                        h1 = pool.tile([128, F], i32, tag=f"h1{prime}")
                        nc.vector.tensor_scalar(h1[:], acc[:], MASK, None, Alu.bitwise_and)
                        h0 = pool.tile([128, F], i32, tag=f"h0{prime}")
                        negp = (HASH_MAP_SIZE - prime % HASH_MAP_SIZE) % HASH_MAP_SIZE
                        nc.vector.tensor_scalar(h0[:], acc[:], negp, None, Alu.add)
                        nc.vector.tensor_scalar(h0[:], h0[:], MASK, None, Alu.bitwise_and)
                        dst.extend([h0, h1])
                    xy = pool.tile([128, F], i32, tag="hxy")
                    for dx in range(2):
                        for dy in range(2):
                            nc.vector.tensor_tensor(xy[:], xs[dx][:], hy[dy][:], Alu.bitwise_xor)
                            for dz in range(2):
                                c = 4 * dx + 2 * dy + dz
                                nc.vector.tensor_tensor(idx_l[:, :, c], xy[:], hz[dz][:], Alu.bitwise_xor)
                    # add packed-table row base (values stay < 2^24: exact)
                    idx_flat = idx_l[:].rearrange("p f c -> p (f c)")
                    nc.vector.tensor_scalar(idx_flat, idx_flat, HASH_ROW2_BASE[gs], None, Alu.add)

                    # --- weights w_l [128, F, 8]: w = (wx*wy)*wz ; c = 4dx+2dy+dz ---
                    w_l = pool.tile([128, F, 8], f32, tag="w_l")
                    om = []
                    for d in range(3):
                        o = pool.tile([128, F], f32, tag=f"om{d}")
                        nc.vector.tensor_scalar(o[:], t_f[d][:], -1.0, 1.0, Alu.mult, Alu.add)
                        om.append(o)
                    w01 = pool.tile([128, F], f32, tag="w01")
                    for dx in range(2):
                        wx = t_f[0] if dx else om[0]
                        for dy in range(2):
                            wy = t_f[1] if dy else om[1]
                            nc.vector.tensor_tensor(w01[:], wx[:], wy[:], Alu.mult)
                            for dz in range(2):
                                wz = t_f[2] if dz else om[2]
                                c = 4 * dx + 2 * dy + dz
                                nc.vector.tensor_tensor(w_l[:, :, c], w01[:], wz[:], Alu.mult)

                    # --- gather: F*8 instructions of [128,1] offsets -> bf16 pairs ---
                    v0 = pool.tile([128, F * 8], f32, tag="v0")
                    v1 = pool.tile([128, F * 8], f32, tag="v1")
                    CH = 64

                    def gbody(j_iv):
                        isg = spool.tile([128, CH], i32, tag="isg")
                        vsg = spool.tile([128, CH, 2], bf16, tag="vsg")
                        nc.vector.tensor_copy(isg[:], idx_flat[:, bass.ds(j_iv, CH)])
                        for m in range(CH):
                            nc.gpsimd.indirect_dma_start(
                                out=vsg[:, m, :], out_offset=None, in_=tab2,
                                in_offset=bass.IndirectOffsetOnAxis(ap=isg[:, m:m + 1], axis=0),
                            )
                        nc.scalar.copy(v0[:, bass.ds(j_iv, CH)], vsg[:, :, 0])
                        nc.scalar.copy(v1[:, bass.ds(j_iv, CH)], vsg[:, :, 1])

                    tc.For_i_unrolled(0, F * 8, CH, gbody, max_unroll=2)

                    # --- MAC via mult + reduce over the 8 corners ---
                    prod = pool.tile([128, F, 8], f32, tag="prod")
                    prod_flat = prod[:].rearrange("p f c -> p (f c)")
                    w_flat = w_l[:].rearrange("p f c -> p (f c)")
                    for k, vv in ((0, v0), (1, v1)):
                        nc.vector.tensor_tensor(prod_flat, w_flat, vv[:], Alu.mult)
                        nc.vector.tensor_reduce(oacc[:, :, 2 * li + k], prod[:],
                                                mybir.AxisListType.X, Alu.add)

                # --- per-tile per-partition uint8 quantization ---
                oflat = oacc[:].rearrange("p f k -> p (f k)")
                amax = pool.tile([128, 1], f32, tag="amax")
                nc.vector.tensor_reduce(amax[:], oflat, mybir.AxisListType.X, Alu.max,
                                        apply_absolute_value=True)
                nc.vector.tensor_scalar(amax[:], amax[:], 1e-30, None, Alu.max)
                kk = pool.tile([128, 1], f32, tag="kk")
                nc.vector.reciprocal(kk[:], amax[:])
                nc.vector.tensor_scalar(kk[:], kk[:], 127.0, None, Alu.mult)
                qf = pool.tile([128, F * 30], f32, tag="qf")
                nc.vector.tensor_tensor(qf[:], oflat, kk[:].to_broadcast([128, F * 30]), Alu.mult)
                # f32->uint8 cast is round-to-nearest-even: |err| <= 0.5 step
                nc.vector.tensor_scalar(qf[:], qf[:], 128.0, None, Alu.add)
                q8 = pool.tile([128, F * 30], u8, tag="q8")
                nc.vector.tensor_copy(q8[:], qf[:])
                nc.sync.dma_start(out_v[t_iv, :, :], q8[:])
                nc.sync.dma_start(am_v[t_iv, :, :], amax[:])

            with tc.For_i(0, NT, 1) as t_iv:
                process_tile(t_iv)

            rg = [list(range(NC))]
            nc.gpsimd.collective_compute("AllGather", Alu.bypass, replica_groups=rg,
                                         ins=[oq_local.opt()], outs=[oq_full.opt()])
            nc.gpsimd.collective_compute("AllGather", Alu.bypass, replica_groups=rg,
                                         ins=[am_local.opt()], outs=[am_full.opt()])
            nc.gpsimd.dma_start(outq.ap().rearrange("a b -> (a b)"), oq_full[:])
            nc.gpsimd.dma_start(amax_o.ap().rearrange("a b c -> (a b c)"), am_full[:])

    nc.compile()
    return nc


def _pack_tables(inputs):
    import ml_dtypes
    bf = ml_dtypes.bfloat16
    flat = np.zeros(E_PAD, bf)
    for gs in HASH_GS:
        b = HASH_ROW2_BASE[gs] * 2
        flat[b:b + HASH_MAP_SIZE * 2] = np.asarray(inputs[f'h{gs:04d}'], np.float32).astype(bf).reshape(-1)
    for G in DENSE_GS:
        g = np.asarray(inputs[f'g{G:04d}'], np.float32)
        B = np.empty((G - 1, G - 1, G - 1, 8, 2), np.float32)
        for i in (0, 1):
            for j in (0, 1):
                for k in (0, 1):
                    c = 4 * i + 2 * j + k
                    B[:, :, :, c, :] = g[i:G - 1 + i, j:G - 1 + j, k:G - 1 + k, :]
        b = DENSE_ROW16_BASE[G] * 16
        flat[b:b + (G - 1) ** 3 * 16] = B.astype(bf).reshape(-1)
    return flat


def _make_jit(nc, donate_outputs=True, replicated_outputs=False):
    """Build a cached jitted shard_map callable for a compiled Bass module.

    With replicated_outputs, the NEFF's output tensors hold identical full
    copies on every core (in-kernel AllGather) and the jit outputs use P()
    sharding, so the host can fetch a single device's buffer.
    """
    import jax
    import jax.numpy as jnp
    from jax.sharding import Mesh, PartitionSpec, NamedSharding
    from jax.experimental.shard_map import shard_map
    from concourse import bass2jax, mybir

    bass2jax.install_neuronx_cc_hook()

    partition_name = nc.partition_id_tensor.name if nc.partition_id_tensor else None
    in_names, out_names, out_avals = [], [], []
    for alloc in nc.m.functions[0].allocations:
        if not isinstance(alloc, mybir.MemoryLocationSet):
            continue
        name = alloc.memorylocations[0].name
        if alloc.kind == "ExternalInput":
            if name != partition_name:
                in_names.append(name)
        elif alloc.kind == "ExternalOutput":
            out_names.append(name)
            out_avals.append(jax.core.ShapedArray(tuple(alloc.tensor_shape),
                                                  mybir.dt.np(alloc.dtype)))
    n_params = len(in_names)
    all_in_names = list(in_names) + list(out_names)
    if partition_name is not None:
        all_in_names.append(partition_name)

    def _body(*args):
        operands = list(args)
        if partition_name is not None:
            operands.append(bass2jax.partition_id_tensor())
        outs = bass2jax._bass_exec_p.bind(
            *operands,
            out_avals=tuple(out_avals),
            in_names=tuple(all_in_names),
            out_names=tuple(out_names),
            lowering_input_output_aliases=(),
            sim_require_finite=True,
            sim_require_nnan=True,
            nc=nc,
        )
        return tuple(outs)

    mesh = _S["mesh"]
    n_outs = len(out_names)
    pcore, prep = PartitionSpec("core"), PartitionSpec()
    ospec = prep if replicated_outputs else pcore
    in_specs = (pcore,) * n_params + (ospec,) * n_outs
    out_specs = (ospec,) * n_outs
    donate = tuple(range(n_params, n_params + n_outs)) if donate_outputs else ()
    fn = jax.jit(
        shard_map(_body, mesh=mesh, in_specs=in_specs, out_specs=out_specs,
                  check_rep=False),
        donate_argnums=donate, keep_unused=True,
    )
    zsh = NamedSharding(mesh, ospec)
    if replicated_outputs:
        zero_shapes = [tuple(a.shape) for a in out_avals]
    else:
        zero_shapes = [(NC * a.shape[0], *a.shape[1:]) for a in out_avals]
    zero_dtypes = [a.dtype for a in out_avals]
    zfn = jax.jit(lambda: tuple(jnp.zeros(s, d) for s, d in zip(zero_shapes, zero_dtypes)),
                  out_shardings=tuple(zsh for _ in zero_shapes))
    return fn, zfn, in_names, out_names


def _ensure_built():
    if "jitB" in _S:
        return
    import jax
    from jax.sharding import Mesh, NamedSharding, PartitionSpec
    devices = jax.devices()[:NC]
    assert len(devices) == NC
    _S["mesh"] = Mesh(np.asarray(devices), ("core",))
    _S["ncA"] = _build_gather_nc()
    _S["jitA"], _S["zerosA"], _, _ = _make_jit(_S["ncA"])
    _S["ncB"] = _build_main_nc()
    _S["jitB"], _S["zerosB"], _S["inB"], _S["outB"] = _make_jit(
        _S["ncB"], replicated_outputs=True)
    # persistent pre-faulted dequant buffer: a fresh 240MB numpy allocation
    # page-faults for ~2s on this 1-cpu host, so reuse one across calls
    _S["qf_buf"] = np.zeros((NC, NT, 128, F, 30), np.float32)


def _ensure_tables(inputs):
    import jax
    from jax.sharding import NamedSharding, PartitionSpec
    names = [f'g{gs:04d}' if gs ** 3 <= HASH_MAP_SIZE else f'h{gs:04d}' for gs in GRID_SIZES]
    tabs = [inputs[n] for n in names]
    cached = _S.get("tab_arrays")
    if cached is not None:
        same = all(a is b or np.array_equal(a, b) for a, b in zip(tabs, cached))
        if same:
            return
    flat = _pack_tables(inputs)
    sh = NamedSharding(_S["mesh"], PartitionSpec("core"))
    tshard_g = jax.device_put(flat, sh)
    (zA,) = _S["zerosA"]()
    (tabs_g,) = _S["jitA"](tshard_g, zA)
    tabs_g.block_until_ready()
    _S["tabs_g"] = tabs_g
    _S["tab_arrays"] = [np.asarray(t) for t in tabs]


def kernel(**inputs):
    import os, time
    import jax
    from jax.sharding import NamedSharding, PartitionSpec
    tlog = (lambda msg, t0: print(f"[k] {msg}: {time.time()-t0:.3f}s", flush=True)) \
        if os.environ.get("BASSK_TIME") else (lambda msg, t0: None)

    t0 = time.time()
    _ensure_built()
    tlog("build", t0)
    t0 = time.time()
    _ensure_tables(inputs)
    tlog("tables", t0)

    t0 = time.time()
    z1, z2 = _S["zerosB"]()             # device-side zeros; overlaps x upload
    x = np.ascontiguousarray(np.asarray(inputs["x"], np.float32))
    sh = NamedSharding(_S["mesh"], PartitionSpec("core"))
    x_g = jax.device_put(x, sh)
    tlog("x upload dispatch", t0)
    t0 = time.time()
    outq_g, amax_g = _S["jitB"](x_g, _S["tabs_g"], z1, z2)
    outq_g.block_until_ready()
    tlog("exec", t0)

    t0 = time.time()
    qbuf = outq_g.addressable_shards[0].data
    abuf = amax_g.addressable_shards[0].data
    abuf.copy_to_host_async()
    qbuf.copy_to_host_async()
    am = np.asarray(abuf)               # [NC*NT, 128, 1] f32
    q = np.asarray(qbuf)                # [N, 30] uint8 (single-buffer fetch)
    tlog("fetch", t0)
    t0 = time.time()
    s = (am.reshape(NC, NT, 128) * np.float32(10.0 / 127.0)).astype(np.float32)
    qf = _S["qf_buf"]
    np.copyto(qf, q.reshape(qf.shape), casting="unsafe")
    qf -= np.float32(128.0)
    qf *= s[:, :, :, None, None]
    out = qf.reshape(N, 30)
    tlog("dequant", t0)
    return out


if __name__ == "__main__":
    rng = np.random.default_rng(0)
    ins = {"x": rng.uniform(-2, 2, (N, 3)).astype(np.float32)}
    for gs in GRID_SIZES:
        if gs ** 3 <= HASH_MAP_SIZE:
            ins[f"g{gs:04d}"] = rng.uniform(-1e-5, 1e-5, (gs, gs, gs, 2)).astype(np.float32)
        else:
            ins[f"h{gs:04d}"] = rng.uniform(-1e-5, 1e-5, (HASH_MAP_SIZE, 2)).astype(np.float32)
    import time
    o = kernel(**ins)
    t0 = time.time(); o = kernel(**ins); t1 = time.time()
    print("kernel output", o.shape, o.dtype, float(np.abs(o).max()), f"warm {t1-t0:.2f}s")
